# revision 10
# baseline (speedup 1.0000x reference)
"""Multi-head attention (B=4, N=2048, D=1024, H=16) on 8 TRN2 NeuronCores.

Sharding: core c = (batch b = c // 2, head-group hg = c % 2). Each core:
  - computes Q/K/V for its 8 heads (tensor-parallel slice of qkv_w),
  - runs attention for those heads,
  - computes a partial output projection against its 512 columns of proj_w.
Host sums the two partials per batch and adds biases folded on the host.

Device layouts (all feature-on-partition so that scores come out as
S^T [k, q] with k on partitions — no transposes anywhere):
  xt  [128, 8, 2048]  bf16 : x[b]^T, d = kt*128 + p
  wqk [128, 8, 1024]  bf16 : lhsT for Q (slots 0..3) and K (slots 4..7);
                             slot t covers the head pair (2t, 2t+1), so the
                             PSUM partition j of an output tile = head
                             (2t + j//64), hd = j % 64.
  wv  [128, 8, 512]   bf16 : rhs for V (token-on-partition orientation)
  wp  [128, 4, 1024]  bf16 : lhsT-side contraction layout for the proj
  bqk [128, 8]        f32  : per-feature q/k bias (zero in practice)
  out [2048, 1024]    f32  : partial projection output

V-bias and proj bias are folded host-side: softmax rows sum to 1, so the
V bias contributes exactly proj_w @ v_bias to every output row.
"""

import numpy as np
import ml_dtypes

import concourse.bass as bass
import concourse.tile as tile
from concourse import bacc, library_config, mybir
from concourse._compat import with_exitstack

B, N, D, H, HD = 4, 2048, 1024, 16, 64
NCORES = 8
HPC = 8          # heads per core
FPC = HPC * HD   # 512 features per core
KT = 8           # d-contraction tiles of 128
KTT = 16         # key-token tiles of 128
QB = 512         # q-block size
NQB = N // QB
SCALE = HD ** -0.5

F32 = mybir.dt.float32
BF16 = mybir.dt.bfloat16
EXP = mybir.ActivationFunctionType.Exp


def _pbcast(ap, parts):
    """Partition-broadcast AP: read one partition `parts` times (DMA source)."""
    return ap.partition_broadcast(parts)


@with_exitstack
def _attn_body(ctx, tc, xt_d, wqk_d, wv_d, wp_d, bqk_d, out_d):
    nc = tc.nc

    singles = ctx.enter_context(tc.tile_pool(name="singles", bufs=1))
    evac = ctx.enter_context(tc.tile_pool(name="evac", bufs=3))
    ppool = ctx.enter_context(tc.tile_pool(name="ppool", bufs=3))
    rpool = ctx.enter_context(tc.tile_pool(name="rpool", bufs=3))
    ps_a = ctx.enter_context(tc.tile_pool(name="ps_a", bufs=2, space="PSUM"))
    ps_b = ctx.enter_context(tc.tile_pool(name="ps_b", bufs=2, space="PSUM"))
    ps_c = ctx.enter_context(tc.tile_pool(name="ps_c", bufs=2, space="PSUM"))

    # Resident SBUF tensors.
    xt_sb = singles.tile([128, KT, N], BF16)
    nc.sync.dma_start(xt_sb, xt_d[:])
    wqk_sb = singles.tile([128, KT, 2 * FPC], BF16)
    nc.sync.dma_start(wqk_sb, wqk_d[:])
    wv_sb = singles.tile([128, KT, FPC], BF16)
    nc.sync.dma_start(wv_sb, wv_d[:])
    wp_sb = singles.tile([128, 4, D], BF16)
    nc.sync.dma_start(wp_sb, wp_d[:])
    bqk_sb = singles.tile([128, 8], F32)
    nc.sync.dma_start(bqk_sb, bqk_d[:])
    ones_sb = singles.tile([128, 1], BF16)
    nc.vector.memset(ones_sb, 1.0)
    ones32 = singles.tile([128, 128], F32)
    nc.vector.memset(ones32, 1.0)

    qk_sb = singles.tile([128, 8, N], BF16)          # Q^T slots 0..3, K^T slots 4..7
    v_sb = singles.tile([128, KTT, HPC, HD], BF16)   # V, token-on-partition
    o_sb = singles.tile([128, 4, N], BF16)           # normalized attn out, f-on-partition

    # ---- Phase 1: QKV projection ----
    for ft in range(8):
        for qt in range(4):
            ps = ps_a.tile([128, 512], F32, tag="psA", name="qk_ps")
            for kt in range(KT):
                nc.tensor.matmul(
                    ps,
                    wqk_sb[:, kt, ft * 128:(ft + 1) * 128],
                    xt_sb[:, kt, qt * 512:(qt + 1) * 512],
                    start=(kt == 0), stop=(kt == KT - 1),
                )
            nc.vector.tensor_scalar_add(
                qk_sb[:, ft, qt * 512:(qt + 1) * 512], ps, bqk_sb[:, ft:ft + 1])
    for mt in range(KTT):
        ps = ps_a.tile([128, 512], F32, tag="psA", name="v_ps")
        for kt in range(KT):
            nc.tensor.matmul(
                ps,
                xt_sb[:, kt, mt * 128:(mt + 1) * 128],
                wv_sb[:, kt, :],
                start=(kt == 0), stop=(kt == KT - 1),
            )
        nc.vector.tensor_copy(
            v_sb[:, mt], ps.rearrange("p (h e) -> p h e", h=HPC))

    # ---- Phase 2: attention (head pairs share the PE array) ----
    for t in range(4):
        he, ho = 2 * t, 2 * t + 1
        for qb in range(NQB):
            q0 = qb * QB
            av = ps_b.tile([128, QB], F32, tag="psB", name="av_ps")
            dps = ps_c.tile([128, QB], F32, tag="psC", name="d_ps")
            for kt in range(KTT):
                k0 = kt * 128
                sp = ps_a.tile([128, 1024], F32, tag="psA", name="s_ps")
                # scores S^T for the pair: even head rows 0:64, odd 64:128
                nc.tensor.matmul(
                    sp[:, 0:512],
                    qk_sb[0:64, 4 + t, k0:k0 + 128],
                    qk_sb[0:64, t, q0:q0 + 512],
                    start=True, stop=True,
                )
                nc.tensor.matmul(
                    sp[:, 512:1024],
                    qk_sb[64:128, 4 + t, k0:k0 + 128],
                    qk_sb[64:128, t, q0:q0 + 512],
                    start=True, stop=True,
                )
                pt = ppool.tile([128, 1024], BF16, tag="pt", name="p_t")
                nc.scalar.activation(pt, sp, EXP, scale=SCALE)
                st, fin = (kt == 0), (kt == KTT - 1)
                nc.tensor.matmul(av[0:64, :], v_sb[:, kt, he, :], pt[:, 0:512],
                                 start=st, stop=fin, tile_position=(0, 0), skip_group_check=True)
                nc.tensor.matmul(av[64:128, :], v_sb[:, kt, ho, :], pt[:, 512:1024],
                                 start=st, stop=fin, tile_position=(0, 64), skip_group_check=True)
                nc.tensor.matmul(dps[0:1, :], ones_sb, pt[:, 0:512],
                                 start=st, stop=fin, tile_position=(0, 0), skip_group_check=True)
                nc.tensor.matmul(dps[64:65, :], ones_sb, pt[:, 512:1024],
                                 start=st, stop=fin, tile_position=(0, 64), skip_group_check=True)
            # normalize: o = av * (1 / denom), denom broadcast over partitions
            r = rpool.tile([128, QB], F32, tag="r", name="r_t")
            nc.vector.reciprocal(r[0:1, :], dps[0:1, :])
            nc.vector.reciprocal(r[64:65, :], dps[64:65, :])
            # broadcast 1/denom across partitions with a K=1 ones-matmul
            bc = ps_c.tile([128, QB], F32, tag="psC", name="bc_ps")
            nc.tensor.matmul(bc[0:64, :], ones32[0:1, 0:64], r[0:1, :],
                             start=True, stop=True, skip_group_check=True)
            nc.tensor.matmul(bc[64:128, :], ones32[64:65, 0:64], r[64:65, :],
                             start=True, stop=True, tile_position=(64, 64),
                             skip_group_check=True)
            rb = rpool.tile([128, QB], F32, tag="rb", name="rb_t")
            nc.vector.tensor_copy(rb, bc)
            nc.vector.tensor_mul(o_sb[0:64, t, q0:q0 + QB], av[0:64, :], rb[0:64, :])
            nc.vector.tensor_mul(o_sb[64:128, t, q0:q0 + QB], av[64:128, :], rb[64:128, :])

    # ---- Phase 3: output projection (partial) ----
    for mt in range(KTT):
        for et in range(2):
            ps = ps_b.tile([128, 512], F32, tag="psB", name="pj_ps")
            for t4 in range(4):
                nc.tensor.matmul(
                    ps,
                    o_sb[:, t4, mt * 128:(mt + 1) * 128],
                    wp_sb[:, t4, et * 512:(et + 1) * 512],
                    start=(t4 == 0), stop=(t4 == 3),
                )
            ot = evac.tile([128, 512], F32, tag="oevac", name="o_evac")
            nc.vector.tensor_copy(ot, ps)
            nc.sync.dma_start(
                out_d[mt * 128:(mt + 1) * 128, et * 512:(et + 1) * 512], ot)


def build_nc():
    nc = bacc.Bacc()
    xt = nc.declare_dram_parameter("xt", [128, KT, N], BF16, isOutput=False)
    wqk = nc.declare_dram_parameter("wqk", [128, KT, 2 * FPC], BF16, isOutput=False)
    wv = nc.declare_dram_parameter("wv", [128, KT, FPC], BF16, isOutput=False)
    wp = nc.declare_dram_parameter("wp", [128, 4, D], BF16, isOutput=False)
    bqk = nc.declare_dram_parameter("bqk", [128, 8], F32, isOutput=False)
    out = nc.declare_dram_parameter("out", [N, D], F32, isOutput=True)
    with tile.TileContext(nc) as tc:
        _attn_body(tc, xt, wqk, wv, wp, bqk, out)
    nc.finalize()
    return nc


BF = ml_dtypes.bfloat16


def _lhsT_layout(w, nkt):
    """[K, M] -> [128, nkt, M] with K = kt*128 + p."""
    k, m = w.shape
    assert k == nkt * 128
    return np.ascontiguousarray(
        w.reshape(nkt, 128, m).astype(BF))  # [kt, p, m]


def prep_core_inputs(x, qkv_w, qkv_b, proj_w, c):
    """Build the per-core input map (numpy, final SBUF layouts)."""
    b, hg = divmod(c, 2)
    f0 = hg * FPC
    xt = np.ascontiguousarray(x[b].T)                     # [1024, 2048] f32
    xt_sb = xt.reshape(KT, 128, N).transpose(1, 0, 2)     # [128, 8, 2048]
    wq = qkv_w[f0:f0 + FPC]
    wk = qkv_w[D + f0:D + f0 + FPC]
    wqk = np.concatenate([wq, wk], axis=0)                # [1024, 1024]
    wqk_sb = wqk.T.reshape(KT, 128, 2 * FPC).transpose(1, 0, 2)
    wv = qkv_w[2 * D + f0:2 * D + f0 + FPC]               # [512, 1024]
    wv_sb = wv.T.reshape(KT, 128, FPC).transpose(1, 0, 2)
    wp = proj_w[:, f0:f0 + FPC]                           # [1024e, 512f]
    wp_sb = wp.T.reshape(4, 128, D).transpose(1, 0, 2)
    bqk = np.concatenate(
        [qkv_b[f0:f0 + FPC], qkv_b[D + f0:D + f0 + FPC]]).reshape(8, 128).T
    return {
        "xt": np.ascontiguousarray(xt_sb).astype(BF),
        "wqk": np.ascontiguousarray(wqk_sb).astype(BF),
        "wv": np.ascontiguousarray(wv_sb).astype(BF),
        "wp": np.ascontiguousarray(wp_sb).astype(BF),
        "bqk": np.ascontiguousarray(bqk).astype(np.float32),
    }


def expected_core_out(x, qkv_w, qkv_b, proj_w, c):
    """Numpy model of one core's partial output (for sim debugging)."""
    b, hg = divmod(c, 2)
    f0 = hg * FPC
    xb = x[b].astype(np.float32)
    q = xb @ qkv_w[f0:f0 + FPC].T + qkv_b[f0:f0 + FPC]
    k = xb @ qkv_w[D + f0:D + f0 + FPC].T + qkv_b[D + f0:D + f0 + FPC]
    v = xb @ qkv_w[2 * D + f0:2 * D + f0 + FPC].T          # v-bias folded on host
    out = np.zeros((N, D), np.float32)
    for h in range(HPC):
        qs = q[:, h * HD:(h + 1) * HD]
        ks = k[:, h * HD:(h + 1) * HD]
        vs = v[:, h * HD:(h + 1) * HD]
        s = (qs @ ks.T) * SCALE
        p = np.exp(s - s.max(axis=1, keepdims=True))
        p /= p.sum(axis=1, keepdims=True)
        out += (p @ vs) @ proj_w[:, f0 + h * HD:f0 + (h + 1) * HD].T
    return out


_NC_CACHE = {}


def kernel(x, qkv_w, qkv_b, proj_w, proj_b):
    from concourse.bass_utils import run_bass_kernel_spmd

    x = np.asarray(x, dtype=np.float32)
    qkv_w = np.asarray(qkv_w, dtype=np.float32)
    qkv_b = np.asarray(qkv_b, dtype=np.float32)
    proj_w = np.asarray(proj_w, dtype=np.float32)
    proj_b = np.asarray(proj_b, dtype=np.float32)

    if "nc" not in _NC_CACHE:
        _NC_CACHE["nc"] = build_nc()
    nc = _NC_CACHE["nc"]

    in_maps = [
        prep_core_inputs(x, qkv_w, qkv_b, proj_w, c) for c in range(NCORES)
    ]
    res = run_bass_kernel_spmd(nc, in_maps, core_ids=list(range(NCORES)))
    outs = res.results

    # v-bias folds into a constant row added to every token: proj_w @ v_bias.
    const_row = proj_w @ qkv_b[2 * D:3 * D] + proj_b
    full = np.empty((B, N, D), np.float32)
    for b in range(B):
        full[b] = outs[2 * b]["out"] + outs[2 * b + 1]["out"] + const_row
    return full


# revision 11
# speedup vs baseline: 1.3445x; 1.3445x over previous
"""Multi-head attention (B=4, N=2048, D=1024, H=16) on 8 TRN2 NeuronCores.

Sharding: core c = (batch b = c // 2, head-group hg = c % 2). Each core:
  - computes Q/K/V for its 8 heads (tensor-parallel slice of qkv_w),
  - runs attention for those heads,
  - computes a partial output projection against its 512 columns of proj_w.
Host sums the two partials per batch and adds biases folded on the host.

Device layouts (all feature-on-partition so that scores come out as
S^T [k, q] with k on partitions — no transposes anywhere):
  xt  [128, 8, 2048]  bf16 : x[b]^T, d = kt*128 + p
  wqk [128, 8, 1024]  bf16 : lhsT for Q (slots 0..3) and K (slots 4..7);
                             slot t covers the head pair (2t, 2t+1), so the
                             PSUM partition j of an output tile = head
                             (2t + j//64), hd = j % 64.
  wv  [128, 8, 512]   bf16 : rhs for V (token-on-partition orientation)
  wp  [128, 4, 1024]  bf16 : lhsT-side contraction layout for the proj
  bqk [128, 8]        f32  : per-feature q/k bias (zero in practice)
  out [2048, 1024]    f32  : partial projection output

Attention per head: S^T = K @ Q^T (k on partitions), P = exp(S^T/8) on ACT
(PSUM->SBUF, bf16), O^T_aug = V_aug.T @ P where V_aug = [V | 1] gives the
softmax denominator as row 64 (in-band).  Normalization multiplies by a
reciprocal broadcast across partitions via a DRAM round-trip DMA.

Softmax skips max-subtraction (scores ~N(0,1); exp never overflows fp32).
V-bias and proj bias are folded host-side: softmax rows sum to 1, so the
V bias contributes exactly proj_w @ v_bias to every output row.

Pair t+1's Q/K projection matmuls are interleaved into pair t's attention
loop to keep the TensorE dense (HAM stays at 2.4 GHz) and hide the QKV
phase inside the ACT-bound attention phase.
"""

import numpy as np
import ml_dtypes

import concourse.bass as bass
import concourse.tile as tile
from concourse import bacc, mybir
from concourse._compat import with_exitstack

B, N, D, H, HD = 4, 2048, 1024, 16, 64
NCORES = 8
HPC = 8          # heads per core
FPC = HPC * HD   # 512 features per core
KT = 8           # d-contraction tiles of 128
KTT = 16         # key-token tiles of 128
QB = 512         # q-block size
NQB = N // QB
SCALE = HD ** -0.5

F32 = mybir.dt.float32
BF16 = mybir.dt.bfloat16
EXP = mybir.ActivationFunctionType.Exp


@with_exitstack
def _attn_body(ctx, tc, xt_d, wqk_d, wv_d, wp_d, bqk_d, out_d):
    nc = tc.nc

    singles = ctx.enter_context(tc.tile_pool(name="singles", bufs=1))
    evac = ctx.enter_context(tc.tile_pool(name="evac", bufs=3))
    ppool = ctx.enter_context(tc.tile_pool(name="ppool", bufs=3))
    rpool = ctx.enter_context(tc.tile_pool(name="rpool", bufs=4))
    dpool = ctx.enter_context(tc.tile_pool(name="dpool", bufs=4, space="DRAM"))
    ps_s = ctx.enter_context(tc.tile_pool(name="ps_s", bufs=2, space="PSUM"))
    ps_av = ctx.enter_context(tc.tile_pool(name="ps_av", bufs=3, space="PSUM"))
    ps_w = ctx.enter_context(tc.tile_pool(name="ps_w", bufs=1, space="PSUM"))

    # Resident SBUF tensors.
    xt_sb = singles.tile([128, KT, N], BF16)
    nc.sync.dma_start(xt_sb, xt_d[:])
    wqk_sb = singles.tile([128, KT, 2 * FPC], BF16)
    nc.sync.dma_start(wqk_sb, wqk_d[:])
    wv_sb = singles.tile([128, KT, FPC], BF16)
    nc.sync.dma_start(wv_sb, wv_d[:])
    wp_sb = singles.tile([128, 4, D], BF16)
    nc.sync.dma_start(wp_sb, wp_d[:])
    bqk_sb = singles.tile([128, 8], F32)
    nc.sync.dma_start(bqk_sb, bqk_d[:])

    qk_sb = singles.tile([128, 8, N], BF16)          # Q^T slots 0..3, K^T slots 4..7
    v_sb = singles.tile([128, KTT, HPC, HD + 1], BF16)  # V_aug, token-on-partition
    o_sb = singles.tile([128, 4, N], BF16)           # normalized attn out, f-on-part
    nc.vector.memset(v_sb[:, :, :, HD], 1.0)         # the ones column

    def emit_qk(ft, qt):
        """One (ft, qt) group of the Q/K projection: 8 matmuls + bias evac."""
        ps = ps_w.tile([128, 512], F32, tag="psW", name="qk_ps")
        for kt in range(KT):
            nc.tensor.matmul(
                ps,
                wqk_sb[:, kt, ft * 128:(ft + 1) * 128],
                xt_sb[:, kt, qt * 512:(qt + 1) * 512],
                start=(kt == 0), stop=(kt == KT - 1),
            )
        nc.vector.tensor_scalar_add(
            qk_sb[:, ft, qt * 512:(qt + 1) * 512], ps, bqk_sb[:, ft:ft + 1])

    # ---- Prologue: V for all heads, then Q/K for pair 0 ----
    for mt in range(KTT):
        ps = ps_w.tile([128, 512], F32, tag="psW", name="v_ps")
        for kt in range(KT):
            nc.tensor.matmul(
                ps,
                xt_sb[:, kt, mt * 128:(mt + 1) * 128],
                wv_sb[:, kt, :],
                start=(kt == 0), stop=(kt == KT - 1),
            )
        nc.vector.tensor_copy(
            v_sb[:, mt, :, 0:HD], ps.rearrange("p (h e) -> p h e", h=HPC))
    for qt in range(4):
        emit_qk(0, qt)
        emit_qk(4, qt)

    # ---- Attention; Q/K of pair t+1 interleaved ----
    for t in range(4):
        # (ft, qt) feed order for the next pair, one group per 8 kt-iters
        nxt = [(ft, qt) for qt in range(4) for ft in (t + 1, 4 + t + 1)] \
            if t < 4 - 1 else []
        it = 0
        for qb in range(NQB):
            q0 = qb * QB
            av_e = ps_av.tile([HD + 1, QB], F32, tag="psAV", name="av_e")
            av_o = ps_av.tile([HD + 1, QB], F32, tag="psAV", name="av_o")
            for kt in range(KTT):
                k0 = kt * 128
                sp = ps_s.tile([128, 1024], F32, tag="psS", name="s_ps")
                # scores S^T for the pair: even head rows 0:64, odd 64:128
                nc.tensor.matmul(
                    sp[:, 0:512],
                    qk_sb[0:64, 4 + t, k0:k0 + 128],
                    qk_sb[0:64, t, q0:q0 + 512],
                    start=True, stop=True,
                )
                nc.tensor.matmul(
                    sp[:, 512:1024],
                    qk_sb[64:128, 4 + t, k0:k0 + 128],
                    qk_sb[64:128, t, q0:q0 + 512],
                    start=True, stop=True,
                )
                pt = ppool.tile([128, 1024], BF16, tag="pt", name="p_t")
                nc.scalar.activation(pt, sp, EXP, scale=SCALE)
                st, fin = (kt == 0), (kt == KTT - 1)
                nc.tensor.matmul(av_e, v_sb[:, kt, 2 * t, :], pt[:, 0:512],
                                 start=st, stop=fin)
                nc.tensor.matmul(av_o, v_sb[:, kt, 2 * t + 1, :], pt[:, 512:1024],
                                 start=st, stop=fin)
                if it % 8 == 0 and nxt:
                    ft, qt = nxt.pop(0)
                    emit_qk(ft, qt)
                it += 1
            # normalize: o = av * (1/denom); denom = row 64 of av (ones col)
            r = rpool.tile([128, QB], F32, tag="r", name="r_t")
            nc.vector.reciprocal(r[0:1, :], av_e[HD:HD + 1, :])
            nc.vector.reciprocal(r[64:65, :], av_o[HD:HD + 1, :])
            rd = dpool.tile([2, QB], F32, name="r_dram")
            nc.sync.dma_start(rd[0:1, :], r[0:1, :])
            nc.sync.dma_start(rd[1:2, :], r[64:65, :])
            rb = rpool.tile([128, QB], F32, tag="rb", name="rb_t")
            nc.sync.dma_start(rb[0:64, :], rd[0:1, :].partition_broadcast(64))
            nc.sync.dma_start(rb[64:128, :], rd[1:2, :].partition_broadcast(64))
            nc.vector.tensor_mul(o_sb[0:64, t, q0:q0 + QB], av_e[0:HD, :],
                                 rb[0:64, :])
            ot = rpool.tile([64, QB], BF16, tag="otmp", name="o_tmp")
            nc.vector.tensor_mul(ot, av_o[0:HD, :], rb[64:128, :])
            nc.sync.dma_start(o_sb[64:128, t, q0:q0 + QB], ot)

    # ---- Output projection (partial) ----
    for mt in range(KTT):
        for et in range(2):
            ps = ps_w.tile([128, 512], F32, tag="psW", name="pj_ps")
            for t4 in range(4):
                nc.tensor.matmul(
                    ps,
                    o_sb[:, t4, mt * 128:(mt + 1) * 128],
                    wp_sb[:, t4, et * 512:(et + 1) * 512],
                    start=(t4 == 0), stop=(t4 == 3),
                )
            ot = evac.tile([128, 512], F32, tag="oevac", name="o_evac")
            nc.vector.tensor_copy(ot, ps)
            nc.sync.dma_start(
                out_d[mt * 128:(mt + 1) * 128, et * 512:(et + 1) * 512], ot)


def build_nc():
    nc = bacc.Bacc()
    xt = nc.declare_dram_parameter("xt", [128, KT, N], BF16, isOutput=False)
    wqk = nc.declare_dram_parameter("wqk", [128, KT, 2 * FPC], BF16, isOutput=False)
    wv = nc.declare_dram_parameter("wv", [128, KT, FPC], BF16, isOutput=False)
    wp = nc.declare_dram_parameter("wp", [128, 4, D], BF16, isOutput=False)
    bqk = nc.declare_dram_parameter("bqk", [128, 8], F32, isOutput=False)
    out = nc.declare_dram_parameter("out", [N, D], F32, isOutput=True)
    with tile.TileContext(nc) as tc:
        _attn_body(tc, xt, wqk, wv, wp, bqk, out)
    nc.finalize()
    return nc


BF = ml_dtypes.bfloat16


def prep_core_inputs(x, qkv_w, qkv_b, proj_w, c):
    """Build the per-core input map (numpy, final SBUF layouts)."""
    b, hg = divmod(c, 2)
    f0 = hg * FPC
    xt = np.ascontiguousarray(x[b].T)                     # [1024, 2048] f32
    xt_sb = xt.reshape(KT, 128, N).transpose(1, 0, 2)     # [128, 8, 2048]
    wq = qkv_w[f0:f0 + FPC]
    wk = qkv_w[D + f0:D + f0 + FPC]
    wqk = np.concatenate([wq, wk], axis=0)                # [1024, 1024]
    wqk_sb = wqk.T.reshape(KT, 128, 2 * FPC).transpose(1, 0, 2)
    wv = qkv_w[2 * D + f0:2 * D + f0 + FPC]               # [512, 1024]
    wv_sb = wv.T.reshape(KT, 128, FPC).transpose(1, 0, 2)
    wp = proj_w[:, f0:f0 + FPC]                           # [1024e, 512f]
    wp_sb = wp.T.reshape(4, 128, D).transpose(1, 0, 2)
    bqk = np.concatenate(
        [qkv_b[f0:f0 + FPC], qkv_b[D + f0:D + f0 + FPC]]).reshape(8, 128).T
    return {
        "xt": np.ascontiguousarray(xt_sb).astype(BF),
        "wqk": np.ascontiguousarray(wqk_sb).astype(BF),
        "wv": np.ascontiguousarray(wv_sb).astype(BF),
        "wp": np.ascontiguousarray(wp_sb).astype(BF),
        "bqk": np.ascontiguousarray(bqk).astype(np.float32),
    }


def expected_core_out(x, qkv_w, qkv_b, proj_w, c):
    """Numpy model of one core's partial output (for sim debugging)."""
    b, hg = divmod(c, 2)
    f0 = hg * FPC
    xb = x[b].astype(np.float32)
    q = xb @ qkv_w[f0:f0 + FPC].T + qkv_b[f0:f0 + FPC]
    k = xb @ qkv_w[D + f0:D + f0 + FPC].T + qkv_b[D + f0:D + f0 + FPC]
    v = xb @ qkv_w[2 * D + f0:2 * D + f0 + FPC].T          # v-bias folded on host
    out = np.zeros((N, D), np.float32)
    for h in range(HPC):
        qs = q[:, h * HD:(h + 1) * HD]
        ks = k[:, h * HD:(h + 1) * HD]
        vs = v[:, h * HD:(h + 1) * HD]
        s = (qs @ ks.T) * SCALE
        p = np.exp(s - s.max(axis=1, keepdims=True))
        p /= p.sum(axis=1, keepdims=True)
        out += (p @ vs) @ proj_w[:, f0 + h * HD:f0 + (h + 1) * HD].T
    return out


_NC_CACHE = {}


def kernel(x, qkv_w, qkv_b, proj_w, proj_b):
    from concourse.bass_utils import run_bass_kernel_spmd

    x = np.asarray(x, dtype=np.float32)
    qkv_w = np.asarray(qkv_w, dtype=np.float32)
    qkv_b = np.asarray(qkv_b, dtype=np.float32)
    proj_w = np.asarray(proj_w, dtype=np.float32)
    proj_b = np.asarray(proj_b, dtype=np.float32)

    if "nc" not in _NC_CACHE:
        _NC_CACHE["nc"] = build_nc()
    nc = _NC_CACHE["nc"]

    in_maps = [
        prep_core_inputs(x, qkv_w, qkv_b, proj_w, c) for c in range(NCORES)
    ]
    res = run_bass_kernel_spmd(nc, in_maps, core_ids=list(range(NCORES)))
    outs = res.results

    # v-bias folds into a constant row added to every token: proj_w @ v_bias.
    const_row = proj_w @ qkv_b[2 * D:3 * D] + proj_b
    full = np.empty((B, N, D), np.float32)
    for b in range(B):
        full[b] = outs[2 * b]["out"] + outs[2 * b + 1]["out"] + const_row
    return full


# revision 15
# speedup vs baseline: 1.3472x; 1.0020x over previous
"""Multi-head attention (B=4, N=2048, D=1024, H=16) on 8 TRN2 NeuronCores.

Sharding: core c = (batch b = c // 2, head-group hg = c % 2). Each core:
  - computes Q/K/V for its 8 heads (tensor-parallel slice of qkv_w),
  - runs attention for those heads,
  - computes a partial output projection against its 512 columns of proj_w.
Host sums the two partials per batch and adds biases folded on the host.

Device layouts (all feature-on-partition so that scores come out as
S^T [k, q] with k on partitions — no transposes anywhere):
  xt  [128, 8, 2048]  bf16 : x[b]^T, d = kt*128 + p
  wqk [128, 8, 1024]  bf16 : lhsT for Q (slots 0..3) and K (slots 4..7);
                             slot t covers the head pair (2t, 2t+1), so the
                             PSUM partition j of an output tile = head
                             (2t + j//64), hd = j % 64.
  wv  [128, 8, 512]   bf16 : rhs for V (token-on-partition orientation)
  wp  [128, 4, 1024]  bf16 : lhsT-side contraction layout for the proj
  bqk [128, 8]        f32  : per-feature q/k bias (zero in practice)
  out [2048, 1024]    f32  : partial projection output

Attention per head: S^T = K @ Q^T (k on partitions), P = exp(S^T/8) on ACT
(PSUM->SBUF, bf16), O^T_aug = V_aug.T @ P where V_aug = [V | 1] gives the
softmax denominator as row 64 (in-band).  Normalization multiplies by a
reciprocal broadcast across partitions via a DRAM round-trip DMA.

Softmax skips max-subtraction (scores ~N(0,1); exp never overflows fp32).
V-bias and proj bias are folded host-side: softmax rows sum to 1, so the
V bias contributes exactly proj_w @ v_bias to every output row.

Pair t+1's Q/K projection matmuls are interleaved into pair t's attention
loop to keep the TensorE dense (HAM stays at 2.4 GHz) and hide the QKV
phase inside the ACT-bound attention phase.
"""

import numpy as np
import ml_dtypes

import concourse.bass as bass
import concourse.tile as tile
from concourse import bacc, mybir
from concourse._compat import with_exitstack

B, N, D, H, HD = 4, 2048, 1024, 16, 64
NCORES = 8
HPC = 8          # heads per core
FPC = HPC * HD   # 512 features per core
KT = 8           # d-contraction tiles of 128
KTT = 16         # key-token tiles of 128
QB = 512         # q-block size
NQB = N // QB
SCALE = HD ** -0.5

F32 = mybir.dt.float32
BF16 = mybir.dt.bfloat16
EXP = mybir.ActivationFunctionType.Exp


@with_exitstack
def _attn_body(ctx, tc, xt_d, wqk_d, wv_d, wp_d, bqk_d, out_d):
    nc = tc.nc

    singles = ctx.enter_context(tc.tile_pool(name="singles", bufs=1))
    evac = ctx.enter_context(tc.tile_pool(name="evac", bufs=3))
    ppool = ctx.enter_context(tc.tile_pool(name="ppool", bufs=3))
    rpool = ctx.enter_context(tc.tile_pool(name="rpool", bufs=4))
    ps_s = ctx.enter_context(tc.tile_pool(name="ps_s", bufs=2, space="PSUM"))
    ps_av = ctx.enter_context(tc.tile_pool(name="ps_av", bufs=3, space="PSUM"))
    ps_w = ctx.enter_context(tc.tile_pool(name="ps_w", bufs=1, space="PSUM"))

    # Resident SBUF tensors.
    xt_sb = singles.tile([128, KT, N], BF16)
    nc.sync.dma_start(xt_sb, xt_d[:])
    wqk_sb = singles.tile([128, KT, 2 * FPC], BF16)
    nc.sync.dma_start(wqk_sb, wqk_d[:])
    wv_sb = singles.tile([128, KT, FPC], BF16)
    nc.sync.dma_start(wv_sb, wv_d[:])
    wp_sb = singles.tile([128, 4, D], BF16)
    nc.sync.dma_start(wp_sb, wp_d[:])
    bqk_sb = singles.tile([128, 8], F32)
    nc.sync.dma_start(bqk_sb, bqk_d[:])

    qk_sb = singles.tile([128, 8, N], BF16)          # Q^T slots 0..3, K^T slots 4..7
    v_sb = singles.tile([128, KTT, HPC, HD + 1], BF16)  # V_aug, token-on-partition
    o_sb = singles.tile([128, 4, N], BF16)           # normalized attn out, f-on-part
    nc.vector.memset(v_sb[:, :, :, HD], 1.0)         # the ones column
    ones_bf = singles.tile([128, 64], BF16)
    nc.vector.memset(ones_bf, 1.0)

    def emit_qk(ft, qt):
        """One (ft, qt) group of the Q/K projection: 8 matmuls + bias evac."""
        ps = ps_w.tile([128, 512], F32, tag="psW", name="qk_ps")
        for kt in range(KT):
            nc.tensor.matmul(
                ps,
                wqk_sb[:, kt, ft * 128:(ft + 1) * 128],
                xt_sb[:, kt, qt * 512:(qt + 1) * 512],
                start=(kt == 0), stop=(kt == KT - 1),
            )
        nc.vector.tensor_scalar_add(
            qk_sb[:, ft, qt * 512:(qt + 1) * 512], ps, bqk_sb[:, ft:ft + 1])

    # ---- Prologue: V for all heads, then Q/K for pair 0 ----
    for mt in range(KTT):
        ps = ps_w.tile([128, 512], F32, tag="psW", name="v_ps")
        for kt in range(KT):
            nc.tensor.matmul(
                ps,
                xt_sb[:, kt, mt * 128:(mt + 1) * 128],
                wv_sb[:, kt, :],
                start=(kt == 0), stop=(kt == KT - 1),
            )
        nc.vector.tensor_copy(
            v_sb[:, mt, :, 0:HD], ps.rearrange("p (h e) -> p h e", h=HPC))
    for qt in range(4):
        emit_qk(0, qt)
        emit_qk(4, qt)

    # ---- Attention; Q/K of pair t+1 interleaved ----
    for t in range(4):
        # (ft, qt) feed order for the next pair, one group per 8 kt-iters
        nxt = [(ft, qt) for qt in range(4) for ft in (t + 1, 4 + t + 1)] \
            if t < 4 - 1 else []
        it = 0
        for qb in range(NQB):
            q0 = qb * QB
            av_e = ps_av.tile([HD + 1, QB], F32, tag="psAV", name="av_e")
            av_o = ps_av.tile([HD + 1, QB], F32, tag="psAV", name="av_o")
            for kt in range(KTT):
                k0 = kt * 128
                sp = ps_s.tile([128, 1024], F32, tag="psS", name="s_ps")
                # scores S^T for the pair: even head rows 0:64, odd 64:128
                nc.tensor.matmul(
                    sp[:, 0:512],
                    qk_sb[0:64, 4 + t, k0:k0 + 128],
                    qk_sb[0:64, t, q0:q0 + 512],
                    start=True, stop=True,
                )
                nc.tensor.matmul(
                    sp[:, 512:1024],
                    qk_sb[64:128, 4 + t, k0:k0 + 128],
                    qk_sb[64:128, t, q0:q0 + 512],
                    start=True, stop=True,
                )
                pt = ppool.tile([128, 1024], BF16, tag="pt", name="p_t")
                nc.scalar.activation(pt, sp, EXP, scale=SCALE)
                st, fin = (kt == 0), (kt == KTT - 1)
                nc.tensor.matmul(av_e, v_sb[:, kt, 2 * t, :], pt[:, 0:512],
                                 start=st, stop=fin)
                nc.tensor.matmul(av_o, v_sb[:, kt, 2 * t + 1, :], pt[:, 512:1024],
                                 start=st, stop=fin)
                if it % 8 == 0 and nxt:
                    ft, qt = nxt.pop(0)
                    emit_qk(ft, qt)
                it += 1
            # normalize: o = av * (1/denom); denom = row 64 of av (ones col).
            # The reciprocal is broadcast across partitions with a K=1
            # ones-matmul (bf16), borrowing a scores PSUM slot.
            r = rpool.tile([128, QB], BF16, tag="r", name="r_t")
            with nc.allow_low_precision(reason="softmax denom reciprocal in bf16"):
                nc.vector.reciprocal(r[0:1, :], av_e[HD:HD + 1, :])
                nc.vector.reciprocal(r[64:65, :], av_o[HD:HD + 1, :])
            bc = ps_s.tile([128, 1024], F32, tag="psS", name="bc_ps")
            nc.tensor.matmul(bc[0:64, 0:512], ones_bf[0:1, :], r[0:1, :],
                             start=True, stop=True)
            nc.tensor.matmul(bc[64:128, 512:1024], ones_bf[64:65, :],
                             r[64:65, :], start=True, stop=True)
            rb = rpool.tile([128, QB], F32, tag="rb", name="rb_t")
            nc.vector.tensor_copy(rb[0:64, :], bc[0:64, 0:512])
            nc.vector.tensor_copy(rb[64:128, :], bc[64:128, 512:1024])
            nc.vector.tensor_mul(o_sb[0:64, t, q0:q0 + QB], av_e[0:HD, :],
                                 rb[0:64, :])
            ot = rpool.tile([64, QB], BF16, tag="otmp", name="o_tmp")
            nc.vector.tensor_mul(ot, av_o[0:HD, :], rb[64:128, :])
            nc.sync.dma_start(o_sb[64:128, t, q0:q0 + QB], ot)

    # ---- Output projection (partial) ----
    for mt in range(KTT):
        for et in range(2):
            ps = ps_w.tile([128, 512], F32, tag="psW", name="pj_ps")
            for t4 in range(4):
                nc.tensor.matmul(
                    ps,
                    o_sb[:, t4, mt * 128:(mt + 1) * 128],
                    wp_sb[:, t4, et * 512:(et + 1) * 512],
                    start=(t4 == 0), stop=(t4 == 3),
                )
            ot = evac.tile([128, 512], F32, tag="oevac", name="o_evac")
            nc.vector.tensor_copy(ot, ps)
            nc.sync.dma_start(
                out_d[mt * 128:(mt + 1) * 128, et * 512:(et + 1) * 512], ot)


def build_nc():
    nc = bacc.Bacc()
    xt = nc.declare_dram_parameter("xt", [128, KT, N], BF16, isOutput=False)
    wqk = nc.declare_dram_parameter("wqk", [128, KT, 2 * FPC], BF16, isOutput=False)
    wv = nc.declare_dram_parameter("wv", [128, KT, FPC], BF16, isOutput=False)
    wp = nc.declare_dram_parameter("wp", [128, 4, D], BF16, isOutput=False)
    bqk = nc.declare_dram_parameter("bqk", [128, 8], F32, isOutput=False)
    out = nc.declare_dram_parameter("out", [N, D], F32, isOutput=True)
    with tile.TileContext(nc) as tc:
        _attn_body(tc, xt, wqk, wv, wp, bqk, out)
    nc.finalize()
    return nc


BF = ml_dtypes.bfloat16


def prep_core_inputs(x, qkv_w, qkv_b, proj_w, c):
    """Build the per-core input map (numpy, final SBUF layouts)."""
    b, hg = divmod(c, 2)
    f0 = hg * FPC
    xt = np.ascontiguousarray(x[b].T)                     # [1024, 2048] f32
    xt_sb = xt.reshape(KT, 128, N).transpose(1, 0, 2)     # [128, 8, 2048]
    wq = qkv_w[f0:f0 + FPC]
    wk = qkv_w[D + f0:D + f0 + FPC]
    wqk = np.concatenate([wq, wk], axis=0)                # [1024, 1024]
    wqk_sb = wqk.T.reshape(KT, 128, 2 * FPC).transpose(1, 0, 2)
    wv = qkv_w[2 * D + f0:2 * D + f0 + FPC]               # [512, 1024]
    wv_sb = wv.T.reshape(KT, 128, FPC).transpose(1, 0, 2)
    wp = proj_w[:, f0:f0 + FPC]                           # [1024e, 512f]
    wp_sb = wp.T.reshape(4, 128, D).transpose(1, 0, 2)
    bqk = np.concatenate(
        [qkv_b[f0:f0 + FPC], qkv_b[D + f0:D + f0 + FPC]]).reshape(8, 128).T
    return {
        "xt": np.ascontiguousarray(xt_sb).astype(BF),
        "wqk": np.ascontiguousarray(wqk_sb).astype(BF),
        "wv": np.ascontiguousarray(wv_sb).astype(BF),
        "wp": np.ascontiguousarray(wp_sb).astype(BF),
        "bqk": np.ascontiguousarray(bqk).astype(np.float32),
    }


def expected_core_out(x, qkv_w, qkv_b, proj_w, c):
    """Numpy model of one core's partial output (for sim debugging)."""
    b, hg = divmod(c, 2)
    f0 = hg * FPC
    xb = x[b].astype(np.float32)
    q = xb @ qkv_w[f0:f0 + FPC].T + qkv_b[f0:f0 + FPC]
    k = xb @ qkv_w[D + f0:D + f0 + FPC].T + qkv_b[D + f0:D + f0 + FPC]
    v = xb @ qkv_w[2 * D + f0:2 * D + f0 + FPC].T          # v-bias folded on host
    out = np.zeros((N, D), np.float32)
    for h in range(HPC):
        qs = q[:, h * HD:(h + 1) * HD]
        ks = k[:, h * HD:(h + 1) * HD]
        vs = v[:, h * HD:(h + 1) * HD]
        s = (qs @ ks.T) * SCALE
        p = np.exp(s - s.max(axis=1, keepdims=True))
        p /= p.sum(axis=1, keepdims=True)
        out += (p @ vs) @ proj_w[:, f0 + h * HD:f0 + (h + 1) * HD].T
    return out


_NC_CACHE = {}


def kernel(x, qkv_w, qkv_b, proj_w, proj_b):
    from concourse.bass_utils import run_bass_kernel_spmd

    x = np.asarray(x, dtype=np.float32)
    qkv_w = np.asarray(qkv_w, dtype=np.float32)
    qkv_b = np.asarray(qkv_b, dtype=np.float32)
    proj_w = np.asarray(proj_w, dtype=np.float32)
    proj_b = np.asarray(proj_b, dtype=np.float32)

    if "nc" not in _NC_CACHE:
        _NC_CACHE["nc"] = build_nc()
    nc = _NC_CACHE["nc"]

    in_maps = [
        prep_core_inputs(x, qkv_w, qkv_b, proj_w, c) for c in range(NCORES)
    ]
    res = run_bass_kernel_spmd(nc, in_maps, core_ids=list(range(NCORES)))
    outs = res.results

    # v-bias folds into a constant row added to every token: proj_w @ v_bias.
    const_row = proj_w @ qkv_b[2 * D:3 * D] + proj_b
    full = np.empty((B, N, D), np.float32)
    for b in range(B):
        full[b] = outs[2 * b]["out"] + outs[2 * b + 1]["out"] + const_row
    return full


# revision 19
# speedup vs baseline: 1.6222x; 1.2041x over previous
"""Multi-head attention (B=4, N=2048, D=1024, H=16) on 8 TRN2 NeuronCores.

Sharding: core c = (batch b = c // 2, head-group hg = c % 2). Each core:
  - computes Q/K/V for its 8 heads (tensor-parallel slice of qkv_w),
  - runs attention for those heads,
  - computes a partial output projection against its 512 columns of proj_w.
Host sums the two partials per batch and adds biases folded on the host.

Device layouts (all feature-on-partition so that scores come out as
S^T [k, q] with k on partitions — no transposes anywhere):
  xt  [128, 8, 2048]  bf16 : x[b]^T, d = kt*128 + p
  wqk [128, 8, 1024]  bf16 : lhsT for Q (slots 0..3) and K (slots 4..7);
                             slot t covers the head pair (2t, 2t+1), so the
                             PSUM partition j of an output tile = head
                             (2t + j//64), hd = j % 64.
  wv  [128, 8, 512]   bf16 : rhs for V (token-on-partition orientation)
  wp  [128, 4, 1024]  bf16 : lhsT-side contraction layout for the proj
  bqk [128, 8]        f32  : per-feature q/k bias (zero in practice)
  out [2048, 1024]    f32  : partial projection output

Attention per head: S^T = K @ Q^T (k on partitions), P = exp(S^T/8) on ACT
(PSUM->SBUF, bf16), O^T_aug = V_aug.T @ P where V_aug = [V | 1] gives the
softmax denominator as row 64 (in-band).  Normalization multiplies by a
reciprocal broadcast across partitions via a DRAM round-trip DMA.

Softmax skips max-subtraction (scores ~N(0,1); exp never overflows fp32).
V-bias and proj bias are folded host-side: softmax rows sum to 1, so the
V bias contributes exactly proj_w @ v_bias to every output row.

Pair t+1's Q/K projection matmuls are interleaved into pair t's attention
loop to keep the TensorE dense (HAM stays at 2.4 GHz) and hide the QKV
phase inside the ACT-bound attention phase.
"""

import numpy as np
import ml_dtypes

import concourse.bass as bass
import concourse.tile as tile
from concourse import bacc, mybir
from concourse._compat import with_exitstack

B, N, D, H, HD = 4, 2048, 1024, 16, 64
NCORES = 8
HPC = 8          # heads per core
FPC = HPC * HD   # 512 features per core
KT = 8           # d-contraction tiles of 128
KTT = 16         # key-token tiles of 128
QB = 512         # q-block size
NQB = N // QB
SCALE = HD ** -0.5

F32 = mybir.dt.float32
BF16 = mybir.dt.bfloat16
EXP = mybir.ActivationFunctionType.Exp


@with_exitstack
def _attn_body(ctx, tc, xt_d, wqk_d, wv_d, wp_d, bqk_d, out_d):
    nc = tc.nc

    singles = ctx.enter_context(tc.tile_pool(name="singles", bufs=1))
    evac = ctx.enter_context(tc.tile_pool(name="evac", bufs=3))
    ppool = ctx.enter_context(tc.tile_pool(name="ppool", bufs=3))
    rpool = ctx.enter_context(tc.tile_pool(name="rpool", bufs=4))
    dpool = ctx.enter_context(tc.tile_pool(name="dpool", bufs=6, space="DRAM"))
    ps_s = ctx.enter_context(tc.tile_pool(name="ps_s", bufs=2, space="PSUM"))
    ps_av = ctx.enter_context(tc.tile_pool(name="ps_av", bufs=3, space="PSUM"))
    ps_w = ctx.enter_context(tc.tile_pool(name="ps_w", bufs=1, space="PSUM"))

    # Resident SBUF tensors.
    xt_sb = singles.tile([128, KT, N], BF16)
    nc.sync.dma_start(xt_sb, xt_d[:])
    wqk_sb = singles.tile([128, KT, 2 * FPC], BF16)
    nc.sync.dma_start(wqk_sb, wqk_d[:])
    wv_sb = singles.tile([128, KT, FPC], BF16)
    nc.sync.dma_start(wv_sb, wv_d[:])
    wp_sb = singles.tile([128, 4, D], BF16)
    nc.sync.dma_start(wp_sb, wp_d[:])
    bqk_sb = singles.tile([128, 8], F32)
    nc.sync.dma_start(bqk_sb, bqk_d[:])

    qk_sb = singles.tile([128, 8, N], BF16)          # Q^T slots 0..3, K^T slots 4..7
    v_sb = singles.tile([128, KTT, HPC, HD + 1], BF16)  # V_aug, token-on-partition
    o_sb = singles.tile([128, 4, N], BF16)           # normalized attn out, f-on-part
    nc.vector.memset(v_sb[:, :, :, HD], 1.0)         # the ones column

    def emit_qk(ft, qt):
        """One (ft, qt) group of the Q/K projection: 8 matmuls + bias evac."""
        ps = ps_w.tile([128, 512], F32, tag="psW", name="qk_ps")
        for kt in range(KT):
            nc.tensor.matmul(
                ps,
                wqk_sb[:, kt, ft * 128:(ft + 1) * 128],
                xt_sb[:, kt, qt * 512:(qt + 1) * 512],
                start=(kt == 0), stop=(kt == KT - 1),
            )
        nc.vector.tensor_scalar_add(
            qk_sb[:, ft, qt * 512:(qt + 1) * 512], ps, bqk_sb[:, ft:ft + 1])

    # ---- Prologue: V for all heads, then Q/K for pair 0 ----
    for mt in range(KTT):
        ps = ps_w.tile([128, 512], F32, tag="psW", name="v_ps")
        for kt in range(KT):
            nc.tensor.matmul(
                ps,
                xt_sb[:, kt, mt * 128:(mt + 1) * 128],
                wv_sb[:, kt, :],
                start=(kt == 0), stop=(kt == KT - 1),
            )
        nc.vector.tensor_copy(
            v_sb[:, mt, :, 0:HD], ps.rearrange("p (h e) -> p h e", h=HPC))
    for qt in range(4):
        emit_qk(0, qt)
        emit_qk(4, qt)

    # ---- Attention; Q/K of pair t+1 interleaved ----
    for t in range(4):
        # (ft, qt) feed order for the next pair, one group per 8 kt-iters
        nxt = [(ft, qt) for qt in range(4) for ft in (t + 1, 4 + t + 1)] \
            if t < 4 - 1 else []
        it = 0
        for qb in range(NQB):
            q0 = qb * QB
            av_e = ps_av.tile([HD + 1, QB], F32, tag="psAV", name="av_e")
            av_o = ps_av.tile([HD + 1, QB], F32, tag="psAV", name="av_o")
            for kt in range(KTT):
                k0 = kt * 128
                sp = ps_s.tile([128, 1024], F32, tag="psS", name="s_ps")
                # scores S^T for the pair: even head rows 0:64, odd 64:128
                nc.tensor.matmul(
                    sp[:, 0:512],
                    qk_sb[0:64, 4 + t, k0:k0 + 128],
                    qk_sb[0:64, t, q0:q0 + 512],
                    start=True, stop=True,
                )
                nc.tensor.matmul(
                    sp[:, 512:1024],
                    qk_sb[64:128, 4 + t, k0:k0 + 128],
                    qk_sb[64:128, t, q0:q0 + 512],
                    start=True, stop=True,
                )
                pt = ppool.tile([128, 1024], BF16, tag="pt", name="p_t")
                nc.scalar.activation(pt, sp, EXP, scale=SCALE)
                st, fin = (kt == 0), (kt == KTT - 1)
                nc.tensor.matmul(av_e, v_sb[:, kt, 2 * t, :], pt[:, 0:512],
                                 start=st, stop=fin)
                nc.tensor.matmul(av_o, v_sb[:, kt, 2 * t + 1, :], pt[:, 512:1024],
                                 start=st, stop=fin)
                if it % 8 == 0 and nxt:
                    ft, qt = nxt.pop(0)
                    emit_qk(ft, qt)
                it += 1
            # Evacuate AV to SBUF right away so the PSUM tiles free up and the
            # normalize chain runs fully off the critical path.
            avs = rpool.tile([HD + 1, 2 * QB], F32, tag="avs", name="avs_t")
            nc.vector.tensor_copy(avs[:, 0:QB], av_e)
            nc.vector.tensor_copy(avs[:, QB:2 * QB], av_o)
            # normalize: o = av * (1/denom); denom = row 64 of avs (ones col).
            # Reciprocal broadcast across partitions via a DRAM round trip.
            r = rpool.tile([128, 2 * QB], F32, tag="r", name="r_t")
            nc.vector.reciprocal(r[HD:HD + 1, :], avs[HD:HD + 1, :])
            rd = dpool.tile([2, QB], F32, name="r_dram")
            nc.sync.dma_start(rd[0:1, :], r[HD:HD + 1, 0:QB])
            nc.sync.dma_start(rd[1:2, :], r[HD:HD + 1, QB:2 * QB])
            rb = rpool.tile([64, 2 * QB], F32, tag="rb", name="rb_t")
            nc.sync.dma_start(rb[:, 0:QB], rd[0:1, :].partition_broadcast(64))
            nc.sync.dma_start(rb[:, QB:2 * QB],
                              rd[1:2, :].partition_broadcast(64))
            nc.vector.tensor_mul(o_sb[0:64, t, q0:q0 + QB], avs[0:HD, 0:QB],
                                 rb[:, 0:QB])
            ot = rpool.tile([64, QB], BF16, tag="otmp", name="o_tmp")
            nc.vector.tensor_mul(ot, avs[0:HD, QB:2 * QB], rb[:, QB:2 * QB])
            nc.sync.dma_start(o_sb[64:128, t, q0:q0 + QB], ot)

    # ---- Output projection (partial) ----
    for mt in range(KTT):
        for et in range(2):
            ps = ps_w.tile([128, 512], F32, tag="psW", name="pj_ps")
            for t4 in range(4):
                nc.tensor.matmul(
                    ps,
                    o_sb[:, t4, mt * 128:(mt + 1) * 128],
                    wp_sb[:, t4, et * 512:(et + 1) * 512],
                    start=(t4 == 0), stop=(t4 == 3),
                )
            ot = evac.tile([128, 512], F32, tag="oevac", name="o_evac")
            nc.vector.tensor_copy(ot, ps)
            nc.sync.dma_start(
                out_d[mt * 128:(mt + 1) * 128, et * 512:(et + 1) * 512], ot)


def build_nc():
    nc = bacc.Bacc()
    xt = nc.declare_dram_parameter("xt", [128, KT, N], BF16, isOutput=False)
    wqk = nc.declare_dram_parameter("wqk", [128, KT, 2 * FPC], BF16, isOutput=False)
    wv = nc.declare_dram_parameter("wv", [128, KT, FPC], BF16, isOutput=False)
    wp = nc.declare_dram_parameter("wp", [128, 4, D], BF16, isOutput=False)
    bqk = nc.declare_dram_parameter("bqk", [128, 8], F32, isOutput=False)
    out = nc.declare_dram_parameter("out", [N, D], F32, isOutput=True)
    with tile.TileContext(nc) as tc:
        _attn_body(tc, xt, wqk, wv, wp, bqk, out)
    nc.finalize()
    return nc


BF = ml_dtypes.bfloat16


def prep_core_inputs(x, qkv_w, qkv_b, proj_w, c):
    """Build the per-core input map (numpy, final SBUF layouts)."""
    b, hg = divmod(c, 2)
    f0 = hg * FPC
    xt = np.ascontiguousarray(x[b].T)                     # [1024, 2048] f32
    xt_sb = xt.reshape(KT, 128, N).transpose(1, 0, 2)     # [128, 8, 2048]
    wq = qkv_w[f0:f0 + FPC]
    wk = qkv_w[D + f0:D + f0 + FPC]
    wqk = np.concatenate([wq, wk], axis=0)                # [1024, 1024]
    wqk_sb = wqk.T.reshape(KT, 128, 2 * FPC).transpose(1, 0, 2)
    wv = qkv_w[2 * D + f0:2 * D + f0 + FPC]               # [512, 1024]
    wv_sb = wv.T.reshape(KT, 128, FPC).transpose(1, 0, 2)
    wp = proj_w[:, f0:f0 + FPC]                           # [1024e, 512f]
    wp_sb = wp.T.reshape(4, 128, D).transpose(1, 0, 2)
    bqk = np.concatenate(
        [qkv_b[f0:f0 + FPC], qkv_b[D + f0:D + f0 + FPC]]).reshape(8, 128).T
    return {
        "xt": np.ascontiguousarray(xt_sb).astype(BF),
        "wqk": np.ascontiguousarray(wqk_sb).astype(BF),
        "wv": np.ascontiguousarray(wv_sb).astype(BF),
        "wp": np.ascontiguousarray(wp_sb).astype(BF),
        "bqk": np.ascontiguousarray(bqk).astype(np.float32),
    }


def expected_core_out(x, qkv_w, qkv_b, proj_w, c):
    """Numpy model of one core's partial output (for sim debugging)."""
    b, hg = divmod(c, 2)
    f0 = hg * FPC
    xb = x[b].astype(np.float32)
    q = xb @ qkv_w[f0:f0 + FPC].T + qkv_b[f0:f0 + FPC]
    k = xb @ qkv_w[D + f0:D + f0 + FPC].T + qkv_b[D + f0:D + f0 + FPC]
    v = xb @ qkv_w[2 * D + f0:2 * D + f0 + FPC].T          # v-bias folded on host
    out = np.zeros((N, D), np.float32)
    for h in range(HPC):
        qs = q[:, h * HD:(h + 1) * HD]
        ks = k[:, h * HD:(h + 1) * HD]
        vs = v[:, h * HD:(h + 1) * HD]
        s = (qs @ ks.T) * SCALE
        p = np.exp(s - s.max(axis=1, keepdims=True))
        p /= p.sum(axis=1, keepdims=True)
        out += (p @ vs) @ proj_w[:, f0 + h * HD:f0 + (h + 1) * HD].T
    return out


_NC_CACHE = {}


def kernel(x, qkv_w, qkv_b, proj_w, proj_b):
    from concourse.bass_utils import run_bass_kernel_spmd

    x = np.asarray(x, dtype=np.float32)
    qkv_w = np.asarray(qkv_w, dtype=np.float32)
    qkv_b = np.asarray(qkv_b, dtype=np.float32)
    proj_w = np.asarray(proj_w, dtype=np.float32)
    proj_b = np.asarray(proj_b, dtype=np.float32)

    if "nc" not in _NC_CACHE:
        _NC_CACHE["nc"] = build_nc()
    nc = _NC_CACHE["nc"]

    in_maps = [
        prep_core_inputs(x, qkv_w, qkv_b, proj_w, c) for c in range(NCORES)
    ]
    res = run_bass_kernel_spmd(nc, in_maps, core_ids=list(range(NCORES)))
    outs = res.results

    # v-bias folds into a constant row added to every token: proj_w @ v_bias.
    const_row = proj_w @ qkv_b[2 * D:3 * D] + proj_b
    full = np.empty((B, N, D), np.float32)
    for b in range(B):
        full[b] = outs[2 * b]["out"] + outs[2 * b + 1]["out"] + const_row
    return full


# revision 24
# speedup vs baseline: 1.6433x; 1.0130x over previous
"""Multi-head attention (B=4, N=2048, D=1024, H=16) on 8 TRN2 NeuronCores.

Sharding: core c = (batch b = c // 2, head-group hg = c % 2). Each core:
  - computes Q/K/V for its 8 heads (tensor-parallel slice of qkv_w),
  - runs attention for those heads,
  - computes a partial output projection against its 512 columns of proj_w.
Host sums the two partials per batch and adds biases folded on the host.

Device layouts (all feature-on-partition so that scores come out as
S^T [k, q] with k on partitions — no transposes anywhere):
  xt  [128, 8, 2048]  bf16 : x[b]^T, d = kt*128 + p
  wqk [128, 8, 1024]  bf16 : lhsT for Q (slots 0..3) and K (slots 4..7);
                             slot t covers the head pair (2t, 2t+1), so the
                             PSUM partition j of an output tile = head
                             (2t + j//64), hd = j % 64.
  wv  [128, 8, 512]   bf16 : rhs for V (token-on-partition orientation)
  wp  [128, 4, 1024]  bf16 : lhsT-side contraction layout for the proj
  bqk [128, 8]        f32  : per-feature q/k bias (zero in practice)
  out [2048, 1024]    f32  : partial projection output

Attention per head: S^T = K @ Q^T (k on partitions), P = exp(S^T/8) on ACT
(PSUM->SBUF, bf16), O^T_aug = V_aug.T @ P where V_aug = [V | 1] gives the
softmax denominator as row 64 (in-band).  Normalization multiplies by a
reciprocal broadcast across partitions via a DRAM round-trip DMA.

Softmax skips max-subtraction (scores ~N(0,1); exp never overflows fp32).
V-bias and proj bias are folded host-side: softmax rows sum to 1, so the
V bias contributes exactly proj_w @ v_bias to every output row.

Pair t+1's Q/K projection matmuls are interleaved into pair t's attention
loop to keep the TensorE dense (HAM stays at 2.4 GHz) and hide the QKV
phase inside the ACT-bound attention phase.
"""

import numpy as np
import ml_dtypes

import concourse.bass as bass
import concourse.tile as tile
from concourse import bacc, mybir
from concourse._compat import with_exitstack

B, N, D, H, HD = 4, 2048, 1024, 16, 64
NCORES = 8
HPC = 8          # heads per core
FPC = HPC * HD   # 512 features per core
KT = 8           # d-contraction tiles of 128
KTT = 16         # key-token tiles of 128
QB = 512         # q-block size
NQB = N // QB
SCALE = HD ** -0.5

F32 = mybir.dt.float32
BF16 = mybir.dt.bfloat16
EXP = mybir.ActivationFunctionType.Exp


@with_exitstack
def _attn_body(ctx, tc, xt_d, wqk_d, wv_d, wp_d, bqk_d, out_d):
    nc = tc.nc

    singles = ctx.enter_context(tc.tile_pool(name="singles", bufs=1))
    evac = ctx.enter_context(tc.tile_pool(name="evac", bufs=3))
    ppool = ctx.enter_context(tc.tile_pool(name="ppool", bufs=3))
    rpool = ctx.enter_context(tc.tile_pool(name="rpool", bufs=4))
    dpool = ctx.enter_context(tc.tile_pool(name="dpool", bufs=6, space="DRAM"))
    ps_s = ctx.enter_context(tc.tile_pool(name="ps_s", bufs=2, space="PSUM"))
    ps_av = ctx.enter_context(tc.tile_pool(name="ps_av", bufs=3, space="PSUM"))
    ps_w = ctx.enter_context(tc.tile_pool(name="ps_w", bufs=1, space="PSUM"))

    # Resident SBUF tensors.
    xt_sb = singles.tile([128, KT, N], BF16)
    nc.sync.dma_start(xt_sb, xt_d[:])
    wqk_sb = singles.tile([128, KT, 2 * FPC], BF16)
    nc.sync.dma_start(wqk_sb, wqk_d[:])
    wv_sb = singles.tile([128, KT, FPC], BF16)
    nc.sync.dma_start(wv_sb, wv_d[:])
    wp_sb = singles.tile([128, 4, D], BF16)
    nc.sync.dma_start(wp_sb, wp_d[:])
    bqk_sb = singles.tile([128, 8], F32)
    nc.sync.dma_start(bqk_sb, bqk_d[:])

    qk_sb = singles.tile([128, 8, N], BF16)          # Q^T slots 0..3, K^T slots 4..7
    v_sb = singles.tile([128, KTT, HPC, HD + 1], BF16)  # V_aug, token-on-partition
    o_sb = singles.tile([128, 4, N], BF16)           # normalized attn out, f-on-part
    nc.vector.memset(v_sb[:, :, :, HD], 1.0)         # the ones column

    def emit_qk(ft, qt, pool=None, tag="psW"):
        """One (ft, qt) group of the Q/K projection: 8 matmuls + bias evac."""
        ps = (pool or ps_w).tile([128, 512], F32, tag=tag, name="qk_ps")
        for kt in range(KT):
            nc.tensor.matmul(
                ps,
                wqk_sb[:, kt, ft * 128:(ft + 1) * 128],
                xt_sb[:, kt, qt * 512:(qt + 1) * 512],
                start=(kt == 0), stop=(kt == KT - 1),
            )
        nc.vector.tensor_scalar_add(
            qk_sb[:, ft, qt * 512:(qt + 1) * 512], ps, bqk_sb[:, ft:ft + 1])

    # ---- Prologue: V for all heads, then Q/K for pair 0 ----
    # (3-buffered PSUM so the 8-matmul groups pipeline with their evacs.)
    for mt in range(KTT):
        ps = ps_av.tile([128, 512], F32, tag="psAV", name="v_ps")
        for kt in range(KT):
            nc.tensor.matmul(
                ps,
                xt_sb[:, kt, mt * 128:(mt + 1) * 128],
                wv_sb[:, kt, :],
                start=(kt == 0), stop=(kt == KT - 1),
            )
        nc.vector.tensor_copy(
            v_sb[:, mt, :, 0:HD], ps.rearrange("p (h e) -> p h e", h=HPC))
    for qt in range(4):
        emit_qk(0, qt, pool=ps_av, tag="psAV")
        emit_qk(4, qt, pool=ps_av, tag="psAV")

    # ---- Attention; Q/K of pair t+1 interleaved ----
    for t in range(4):
        # (ft, qt) feed order for the next pair, one group per 8 kt-iters
        nxt = [(ft, qt) for qt in range(4) for ft in (t + 1, 4 + t + 1)] \
            if t < 4 - 1 else []
        it = 0
        for qb in range(NQB):
            q0 = qb * QB
            av_e = ps_av.tile([HD + 1, QB], F32, tag="psAV", name="av_e")
            av_o = ps_av.tile([HD + 1, QB], F32, tag="psAV", name="av_o")
            for kt in range(KTT):
                k0 = kt * 128
                sp = ps_s.tile([128, 1024], F32, tag="psS", name="s_ps")
                # scores S^T for the pair: even head rows 0:64, odd 64:128
                nc.tensor.matmul(
                    sp[:, 0:512],
                    qk_sb[0:64, 4 + t, k0:k0 + 128],
                    qk_sb[0:64, t, q0:q0 + 512],
                    start=True, stop=True,
                )
                nc.tensor.matmul(
                    sp[:, 512:1024],
                    qk_sb[64:128, 4 + t, k0:k0 + 128],
                    qk_sb[64:128, t, q0:q0 + 512],
                    start=True, stop=True,
                )
                pt = ppool.tile([128, 1024], BF16, tag="pt", name="p_t")
                nc.scalar.activation(pt, sp, EXP, scale=SCALE)
                st, fin = (kt == 0), (kt == KTT - 1)
                nc.tensor.matmul(av_e, v_sb[:, kt, 2 * t, :], pt[:, 0:512],
                                 start=st, stop=fin)
                nc.tensor.matmul(av_o, v_sb[:, kt, 2 * t + 1, :], pt[:, 512:1024],
                                 start=st, stop=fin)
                if it % 8 == 0 and nxt:
                    ft, qt = nxt.pop(0)
                    emit_qk(ft, qt)
                it += 1
            # Evacuate AV to SBUF right away so the PSUM tiles free up and the
            # normalize chain runs fully off the critical path.
            avs = rpool.tile([HD + 1, 2 * QB], F32, tag="avs", name="avs_t")
            nc.vector.tensor_copy(avs[:, 0:QB], av_e)
            nc.vector.tensor_copy(avs[:, QB:2 * QB], av_o)
            # normalize: o = av * (1/denom); denom = row 64 of avs (ones col).
            # DVE reciprocal runs at 1/8 rate, so a single-partition [1,1024]
            # reciprocal costs ~6.5us.  Bounce the denominators through DRAM
            # into a [128, 8] layout first: the reciprocal then uses all 128
            # lanes (~70ns), and the result goes back out as a broadcast.
            rd = dpool.tile([2, QB], F32, name="d_dram")
            nc.sync.dma_start(rd[0:1, :], avs[HD:HD + 1, 0:QB])
            nc.sync.dma_start(rd[1:2, :], avs[HD:HD + 1, QB:2 * QB])
            d128 = rpool.tile([128, 8], F32, tag="d128", name="d128_t")
            nc.sync.dma_start(
                d128, rd[:].rearrange("two (a p) -> p (two a)", p=128))
            r128 = rpool.tile([128, 8], F32, tag="r128", name="r128_t")
            nc.vector.reciprocal(r128, d128)
            rr = dpool.tile([2, QB], F32, name="r_dram")
            nc.sync.dma_start(
                rr[:].rearrange("two (a p) -> p (two a)", p=128), r128)
            rb = rpool.tile([64, 2 * QB], F32, tag="rb", name="rb_t")
            nc.sync.dma_start(rb[:, 0:QB], rr[0:1, :].partition_broadcast(64))
            nc.sync.dma_start(rb[:, QB:2 * QB],
                              rr[1:2, :].partition_broadcast(64))
            nc.vector.tensor_mul(o_sb[0:64, t, q0:q0 + QB], avs[0:HD, 0:QB],
                                 rb[:, 0:QB])
            ot = rpool.tile([64, QB], BF16, tag="otmp", name="o_tmp")
            nc.vector.tensor_mul(ot, avs[0:HD, QB:2 * QB], rb[:, QB:2 * QB])
            nc.sync.dma_start(o_sb[64:128, t, q0:q0 + QB], ot)

    # ---- Output projection (partial) ----
    for mt in range(KTT):
        for et in range(2):
            ps = ps_av.tile([128, 512], F32, tag="psAV", name="pj_ps")
            for t4 in range(4):
                nc.tensor.matmul(
                    ps,
                    o_sb[:, t4, mt * 128:(mt + 1) * 128],
                    wp_sb[:, t4, et * 512:(et + 1) * 512],
                    start=(t4 == 0), stop=(t4 == 3),
                )
            ot = evac.tile([128, 512], F32, tag="oevac", name="o_evac")
            nc.vector.tensor_copy(ot, ps)
            nc.sync.dma_start(
                out_d[mt * 128:(mt + 1) * 128, et * 512:(et + 1) * 512], ot)


def build_nc():
    nc = bacc.Bacc()
    xt = nc.declare_dram_parameter("xt", [128, KT, N], BF16, isOutput=False)
    wqk = nc.declare_dram_parameter("wqk", [128, KT, 2 * FPC], BF16, isOutput=False)
    wv = nc.declare_dram_parameter("wv", [128, KT, FPC], BF16, isOutput=False)
    wp = nc.declare_dram_parameter("wp", [128, 4, D], BF16, isOutput=False)
    bqk = nc.declare_dram_parameter("bqk", [128, 8], F32, isOutput=False)
    out = nc.declare_dram_parameter("out", [N, D], F32, isOutput=True)
    with tile.TileContext(nc) as tc:
        _attn_body(tc, xt, wqk, wv, wp, bqk, out)
    nc.finalize()
    return nc


BF = ml_dtypes.bfloat16


def prep_core_inputs(x, qkv_w, qkv_b, proj_w, c):
    """Build the per-core input map (numpy, final SBUF layouts)."""
    b, hg = divmod(c, 2)
    f0 = hg * FPC
    xt = np.ascontiguousarray(x[b].T)                     # [1024, 2048] f32
    xt_sb = xt.reshape(KT, 128, N).transpose(1, 0, 2)     # [128, 8, 2048]
    wq = qkv_w[f0:f0 + FPC]
    wk = qkv_w[D + f0:D + f0 + FPC]
    wqk = np.concatenate([wq, wk], axis=0)                # [1024, 1024]
    wqk_sb = wqk.T.reshape(KT, 128, 2 * FPC).transpose(1, 0, 2)
    wv = qkv_w[2 * D + f0:2 * D + f0 + FPC]               # [512, 1024]
    wv_sb = wv.T.reshape(KT, 128, FPC).transpose(1, 0, 2)
    wp = proj_w[:, f0:f0 + FPC]                           # [1024e, 512f]
    wp_sb = wp.T.reshape(4, 128, D).transpose(1, 0, 2)
    bqk = np.concatenate(
        [qkv_b[f0:f0 + FPC], qkv_b[D + f0:D + f0 + FPC]]).reshape(8, 128).T
    return {
        "xt": np.ascontiguousarray(xt_sb).astype(BF),
        "wqk": np.ascontiguousarray(wqk_sb).astype(BF),
        "wv": np.ascontiguousarray(wv_sb).astype(BF),
        "wp": np.ascontiguousarray(wp_sb).astype(BF),
        "bqk": np.ascontiguousarray(bqk).astype(np.float32),
    }


def expected_core_out(x, qkv_w, qkv_b, proj_w, c):
    """Numpy model of one core's partial output (for sim debugging)."""
    b, hg = divmod(c, 2)
    f0 = hg * FPC
    xb = x[b].astype(np.float32)
    q = xb @ qkv_w[f0:f0 + FPC].T + qkv_b[f0:f0 + FPC]
    k = xb @ qkv_w[D + f0:D + f0 + FPC].T + qkv_b[D + f0:D + f0 + FPC]
    v = xb @ qkv_w[2 * D + f0:2 * D + f0 + FPC].T          # v-bias folded on host
    out = np.zeros((N, D), np.float32)
    for h in range(HPC):
        qs = q[:, h * HD:(h + 1) * HD]
        ks = k[:, h * HD:(h + 1) * HD]
        vs = v[:, h * HD:(h + 1) * HD]
        s = (qs @ ks.T) * SCALE
        p = np.exp(s - s.max(axis=1, keepdims=True))
        p /= p.sum(axis=1, keepdims=True)
        out += (p @ vs) @ proj_w[:, f0 + h * HD:f0 + (h + 1) * HD].T
    return out


_NC_CACHE = {}


def kernel(x, qkv_w, qkv_b, proj_w, proj_b):
    from concourse.bass_utils import run_bass_kernel_spmd

    x = np.asarray(x, dtype=np.float32)
    qkv_w = np.asarray(qkv_w, dtype=np.float32)
    qkv_b = np.asarray(qkv_b, dtype=np.float32)
    proj_w = np.asarray(proj_w, dtype=np.float32)
    proj_b = np.asarray(proj_b, dtype=np.float32)

    if "nc" not in _NC_CACHE:
        _NC_CACHE["nc"] = build_nc()
    nc = _NC_CACHE["nc"]

    in_maps = [
        prep_core_inputs(x, qkv_w, qkv_b, proj_w, c) for c in range(NCORES)
    ]
    res = run_bass_kernel_spmd(nc, in_maps, core_ids=list(range(NCORES)))
    outs = res.results

    # v-bias folds into a constant row added to every token: proj_w @ v_bias.
    const_row = proj_w @ qkv_b[2 * D:3 * D] + proj_b
    full = np.empty((B, N, D), np.float32)
    for b in range(B):
        full[b] = outs[2 * b]["out"] + outs[2 * b + 1]["out"] + const_row
    return full
